# revision 23
# baseline (speedup 1.0000x reference)
"""CLIP encoder layer (LN -> causal MHA -> residual -> LN -> GELU MLP -> residual)
as a Bass/Tile kernel for Trainium2, data-parallel over batch across 8 NeuronCores.

v2 layout strategy per core (one batch element):
  - matmul path in bf16 (full PE rate, FWL fast weight loads, half the HBM
    traffic); residuals/LN/softmax math in fp32; PSUM accumulation fp32.
  - mask prep done on HOST: transposed, clamped, bf16 slabs only for blocks
    that are partially masked; fully-masked 128x256 score blocks are skipped,
    fully-live blocks need no mask add at all.
  - h kept natural fp32 (residual+LN) and normalized-transposed bf16 (h1T/h2T).
  - attention in scoresT[t_key, s_query] layout, 256-wide query chunks;
    2 heads share the PE via row tiling (kT/qT for the head pair live at
    partitions 0:64 / 64:128 -> tile_position (0,0)/(64,0) auto-derived);
    score blocks for two key tiles share one PSUM bank so exp runs as one
    [128,512] ACT op; softmax denominator = ones column appended to V
    (row 64 of the AV psum); reciprocal via the fast DVE approximation,
    broadcast to [64, q] with a tiny PE matmul.
  - bo/b2 biases pre-added into the residual so projection outputs land with
    a single fused add.
"""

import numpy as np
import ml_dtypes
from contextlib import ExitStack

import concourse.bass as bass
import concourse.mybir as mybir
import concourse.tile as tile
from concourse import bacc
from concourse.bass_utils import run_bass_kernel_spmd
from concourse.masks import make_identity

AF = mybir.ActivationFunctionType
ALU = mybir.AluOpType
F32 = mybir.dt.float32
F32R = mybir.dt.float32r
BF16 = mybir.dt.bfloat16
NPBF16 = ml_dtypes.bfloat16

B, S, E, H, D, F = 8, 1024, 768, 12, 64, 3072
P = 128
ST = S // P            # 8 token tiles
ET = E // P            # 6 embed tiles
FT = F // P            # 24 ffn tiles
QC = 256               # attention query-chunk width
NQC = S // QC          # 4
NP_ = 6                # head pairs
SCALE = float(D) ** -0.5
EPS = 1e-5
MASK_CLAMP = -80.0


def _r(ap):
    """Reinterpret an fp32 AP as float32r for full-rate PE matmuls."""
    return ap.bitcast(F32R)


def _bcast_dma(nc, dst, src_ap):
    """DMA a 1-D DRAM vector to [P, n] SBUF, broadcast along partitions."""
    bsrc = bass.AP(
        tensor=src_ap.tensor, offset=src_ap.offset, ap=[[0, P]] + list(src_ap.ap)
    )
    nc.gpsimd.dma_start(out=dst, in_=bsrc)


def build(kept, needs_add):
    """kept[tk][c]: [128-key x 256-query] score block live for any batch.
    needs_add[tk][c]: block needs an additive mask slab (mixed/partial)."""
    kept = np.asarray(kept, bool)
    needs_add = np.asarray(needs_add, bool)
    nslab = int(needs_add.sum())
    mslot = {}
    for c in range(NQC):
        for tk in range(ST):
            if needs_add[tk][c]:
                mslot[(tk, c)] = len(mslot)

    nc = bacc.Bacc("TRN2", target_bir_lowering=False, debug=False, num_devices=8)

    x_t = nc.dram_tensor("x", [S, E], F32, kind="ExternalInput")
    maskt_t = (
        nc.dram_tensor("maskt", [P, nslab, QC], BF16, kind="ExternalInput")
        if nslab
        else None
    )
    names_1d = ["ln1_g", "ln1_b", "bq", "bk", "bv", "bo", "ln2_g", "ln2_b", "b2"]
    v1 = {n: nc.dram_tensor(n, [E], F32, kind="ExternalInput") for n in names_1d}
    v1["b1"] = nc.dram_tensor("b1", [F], F32, kind="ExternalInput")
    wq_t = nc.dram_tensor("wq", [E, E], BF16, kind="ExternalInput")
    wk_t = nc.dram_tensor("wk", [E, E], BF16, kind="ExternalInput")
    wv_t = nc.dram_tensor("wv", [E, E], BF16, kind="ExternalInput")
    wo_t = nc.dram_tensor("wo", [E, E], BF16, kind="ExternalInput")
    w1_t = nc.dram_tensor("w1", [E, F], BF16, kind="ExternalInput")
    w2_t = nc.dram_tensor("w2", [F, E], BF16, kind="ExternalInput")
    out_t = nc.dram_tensor("out", [S, E], F32, kind="ExternalOutput")

    xa = x_t.ap().rearrange("(n p) e -> p n e", p=P)          # [P, ST, E]
    outa = out_t.ap().rearrange("(n p) e -> p n e", p=P)

    with tile.TileContext(nc) as tc, ExitStack() as top, nc.allow_low_precision(
        reason="bf16 matmul path; accumulation stays fp32 in PSUM"
    ):
        persist = top.enter_context(tc.tile_pool(name="persist", bufs=1))
        psum = top.enter_context(tc.tile_pool(name="psum", bufs=1, space="PSUM"))

        # ---- persistent tiles + small constants ----
        x_h = persist.tile([P, ST, E], F32, name="x_h")       # x, then residual h
        OT = persist.tile([P, ET, S], BF16, name="OT")        # attn out (unnormed->normed), T
        identity = persist.tile([P, P], BF16, name="identity")
        make_identity(nc, identity)
        eps_t = persist.tile([P, 1], F32, name="eps_t")
        nc.vector.memset(eps_t, EPS)
        ones64 = persist.tile([1, 64], F32, name="ones64")
        nc.vector.memset(ones64, 1.0)

        bqs = persist.tile([P, ET], F32, name="bqs")
        bks = persist.tile([P, ET], F32, name="bks")
        b1c = persist.tile([P, FT], F32, name="b1c")
        g1c = persist.tile([P, ET], F32, name="g1c")
        b1cc = persist.tile([P, ET], F32, name="b1cc")
        g2c = persist.tile([P, ET], F32, name="g2c")
        b2cc = persist.tile([P, ET], F32, name="b2cc")
        # spread early DMAs across engine queues: each dma_start costs ~1.1us
        # of issuing-engine time, and a single serial queue starves startup
        nc.gpsimd.dma_start(g1c, v1["ln1_g"].ap().rearrange("(o p) -> p o", p=P))
        nc.gpsimd.dma_start(b1cc, v1["ln1_b"].ap().rearrange("(o p) -> p o", p=P))
        nc.sync.dma_start(bqs, v1["bq"].ap().rearrange("(o p) -> p o", p=P))
        nc.sync.dma_start(bks, v1["bk"].ap().rearrange("(o p) -> p o", p=P))
        nc.sync.dma_start(b1c, v1["b1"].ap().rearrange("(o p) -> p o", p=P))
        nc.sync.dma_start(g2c, v1["ln2_g"].ap().rearrange("(o p) -> p o", p=P))
        nc.sync.dma_start(b2cc, v1["ln2_b"].ap().rearrange("(o p) -> p o", p=P))
        bv_b = persist.tile([P, E], F32, name="bv_b")
        bo_b = persist.tile([P, E], F32, name="bo_b")
        b2b = persist.tile([P, E], F32, name="b2b")
        _bcast_dma(nc, bv_b, v1["bv"].ap())
        _bcast_dma(nc, bo_b, v1["bo"].ap())
        _bcast_dma(nc, b2b, v1["b2"].ap())

        for i in range(ST):  # per-tile so LN1 starts before the full x lands
            nc.sync.dma_start(x_h[:, i, :], xa[:, i, :])

        wo_sb = persist.tile([P, ET, E], BF16, name="wo_sb")
        nc.scalar.dma_start(wo_sb, wo_t.ap().rearrange("(ko p) m -> p ko m", p=P))

        h1T = persist.tile([P, ET, S], BF16, tag="hT", name="h1T")

        U32 = mybir.dt.uint32
        magic = persist.tile([P, ST], U32, name="magic")
        nc.vector.memset(magic, 0x5F3759DF)

        def newton_rsqrt(pool, v, n):
            """DVE-only rstd = v^-1/2 on a [P, n] fp32 tile (quake seed + 2
            Newton iterations, rel err ~1e-6). Keeps ACT free of Ln/Sqrt so
            the exp table stays loaded for the whole attention phase."""
            y = pool.tile([P, n], F32, tag="lnrsy", bufs=2, name="rsy")
            t = pool.tile([P, n], F32, tag="lnrst", bufs=2, name="rst")
            # seed: bits(y) = 0x5F3759DF - (bits(v) >> 1)  (as ~x + C+1)
            nc.vector.tensor_scalar(
                out=y.bitcast(U32),
                in0=v.bitcast(U32),
                scalar1=1,
                scalar2=None,
                op0=ALU.logical_shift_right,
            )
            nc.vector.tensor_tensor(
                out=y.bitcast(U32),
                in0=magic[:, :n],
                in1=y.bitcast(U32),
                op=ALU.subtract,
            )
            for _ in range(2):
                nc.vector.tensor_tensor(out=t, in0=y, in1=y, op=ALU.mult)
                nc.vector.tensor_tensor(out=t, in0=t, in1=v, op=ALU.mult)
                nc.vector.tensor_scalar(
                    out=t, in0=t, scalar1=-0.5, scalar2=1.5, op0=ALU.mult, op1=ALU.add
                )
                nc.vector.tensor_tensor(out=y, in0=y, in1=t, op=ALU.mult)
            return y

        def ln_stats(pool, x_slice, mvs, vs, sl):
            """bn stats for one token tile; mean into mvs[:, sl, :], var+eps
            into vs[:, sl]."""
            xr = x_slice.rearrange("p (n s) -> p n s", s=256)
            stats = pool.tile([P, 3, 6], F32, tag="lnstats", bufs=4, name="stats")
            for sg in range(3):
                nc.vector.bn_stats(out=stats[:, sg, :], in_=xr[:, sg, :])
            nc.vector.bn_aggr(out=mvs[:, sl, :], in_=stats)
            nc.vector.tensor_scalar_add(vs[:, sl : sl + 1], mvs[:, sl, 1:2], EPS)

        def ln_normalize(pool, x_slice, mvs, rs, sl):
            xn = pool.tile([P, E], BF16, tag="lnxn", bufs=3, name="xn")
            nc.vector.tensor_scalar(
                out=xn,
                in0=x_slice,
                scalar1=mvs[:, sl, 0:1],
                scalar2=rs[:, sl : sl + 1],
                op0=ALU.subtract,
                op1=ALU.mult,
            )
            return xn

        def ln_transposes(xn, dstT, i, gc, bc):
            for j in range(ET):
                pt = psum.tile([P, P], BF16, tag="pss", bufs=3, name="pt")
                ptv = pt
                nc.tensor.transpose(ptv, xn[:, j * P : (j + 1) * P], identity)
                nc.scalar.activation(
                    out=dstT[:, j, i * P : (i + 1) * P],
                    in_=ptv,
                    func=AF.Identity,
                    bias=bc[:, j : j + 1],
                    scale=gc[:, j : j + 1],
                )

        # ---------------- LN1 + transpose to h1T ----------------
        with tc.tile_pool(name="ln1p", bufs=1) as lp:
            mvs = lp.tile([P, ST, 2], F32, name="mvs1")
            vs = lp.tile([P, ST], F32, name="vs1")
            for i in range(ST):
                ln_stats(lp, x_h[:, i, :], mvs, vs, i)
            rs = newton_rsqrt(lp, vs, ST)
            for i in range(ST):
                xn = ln_normalize(lp, x_h[:, i, :], mvs, rs, i)
                ln_transposes(xn, h1T, i, g1c, b1cc)

        # ---------------- QKV projections (all 3 groups) ----------------
        qT = persist.tile([P, 3, 2, S], BF16, name="qT")
        kT = persist.tile([P, 3, 2, S], BF16, name="kT")
        vaug = persist.tile([P, 3, ST, 4, D + 1], BF16, name="vaug")
        nc.gpsimd.memset(vaug[:, :, :, :, D : D + 1], 1.0)

        with tc.tile_pool(name="wqkv", bufs=1) as wp:
            wq_sb = wp.tile([P, ET, E], BF16, name="wq_sb")
            wk_sb = wp.tile([P, ET, E], BF16, name="wk_sb")
            wv_sb = wp.tile([P, ET, E], BF16, name="wv_sb")
            nc.scalar.dma_start(wq_sb, wq_t.ap().rearrange("(ko p) m -> p ko m", p=P))
            nc.scalar.dma_start(wk_sb, wk_t.ap().rearrange("(ko p) m -> p ko m", p=P))
            nc.gpsimd.dma_start(wv_sb, wv_t.ap().rearrange("(ko p) m -> p ko m", p=P))

            for g in range(3):
                for w_sb, dstT, bias in ((wq_sb, qT, bqs), (wk_sb, kT, bks)):
                    for jl in range(2):
                        jj = 2 * g + jl
                        for sc2 in range(2):
                            ps = psum.tile([P, 512], F32, tag="pj", bufs=2, name="psq")
                            for ek in range(ET):
                                nc.tensor.matmul(
                                    ps,
                                    w_sb[:, ek, jj * P : (jj + 1) * P],
                                    h1T[:, ek, sc2 * 512 : (sc2 + 1) * 512],
                                    start=(ek == 0),
                                    stop=(ek == ET - 1),
                                )
                            nc.scalar.activation(
                                out=dstT[:, g, jl, sc2 * 512 : (sc2 + 1) * 512],
                                in_=ps,
                                func=AF.Identity,
                                bias=bias[:, jj : jj + 1],
                                scale=1.0,
                            )
                gsl = slice(g * 256, (g + 1) * 256)
                bvr = bv_b[:, gsl].rearrange("p (h d) -> p h d", d=D)
                for i in range(ST):
                    ps = psum.tile([P, 512], F32, tag="pj", bufs=2, name="psv")
                    for ek in range(ET):
                        nc.tensor.matmul(
                            ps[:, :256],
                            h1T[:, ek, i * P : (i + 1) * P],
                            wv_sb[:, ek, gsl],
                            start=(ek == 0),
                            stop=(ek == ET - 1),
                        )
                    nc.vector.tensor_tensor(
                        out=vaug[:, g, i, :, 0:D],
                        in0=ps[:, :256].rearrange("p (h d) -> p h d", d=D),
                        in1=bvr,
                        op=ALU.add,
                    )

        # ---------------- attention + out-proj + LN2 + MLP (interleaved) ----
        h2T = persist.tile([P, ET, S], BF16, tag="hT", name="h2T")
        kept_by_c = {c: [tk for tk in range(ST) if kept[tk][c]] for c in range(NQC)}

        with tc.tile_pool(name="attnp", bufs=1) as ap_, tc.tile_pool(
            name="mlpp", bufs=1
        ) as mp, tc.tile_pool(name="ln2p", bufs=1) as lp2:
            maskt_sb = None
            if nslab:
                maskt_sb = ap_.tile([P, nslab, QC], BF16, name="maskt_sb")
                nc.gpsimd.dma_start(maskt_sb, maskt_t.ap())

            # pre-add bo into the residual (out-proj then lands with one add)
            for i in range(ST):
                nc.vector.tensor_tensor(
                    out=x_h[:, i, :], in0=x_h[:, i, :], in1=bo_b, op=ALU.add
                )

            def attn_unit(p_, c):
                """One head pair x one 256-query chunk."""
                g, jl = divmod(p_, 2)
                tks = kept_by_c[c]
                if not tks:
                    return
                qsl = slice(c * QC, (c + 1) * QC)
                # both heads' AV accumulators share one PSUM bank so three
                # units can be in flight across the 8-bank budget. start=True
                # clears has_written BANK-wide, so the two accumulation chains
                # must NOT interleave: head A's chain runs fully, then head
                # B's (B's start only clears bits; A's data stays readable).
                psa2 = psum.tile([P, 2, QC], F32, tag="pav", bufs=3, name="psa")
                psa = {hx: psa2[:, hx, :] for hx in range(2)}
                pairs = []
                exps = []
                npairs = (len(tks) + 1) // 2
                for tp in range(npairs):
                    pair = tks[2 * tp : 2 * tp + 2]
                    w = len(pair) * QC
                    pss = {}
                    for hx in range(2):
                        ro = hx * 64
                        pss[hx] = psum.tile(
                            [P, 2, QC], F32, tag="pss", bufs=3, name="pss"
                        )
                        for n_, tk in enumerate(pair):
                            nc.tensor.matmul(
                                pss[hx][:, n_, :],
                                kT[ro : ro + 64, g, jl, tk * P : (tk + 1) * P],
                                qT[ro : ro + 64, g, jl, qsl],
                                start=True,
                                stop=True,
                            )
                    adds = [needs_add[tk][c] for tk in pair]
                    merged = (
                        len(pair) == 2
                        and all(adds)
                        and mslot[(pair[1], c)] == mslot[(pair[0], c)] + 1
                    )
                    for hx in range(2):
                        if merged:
                            m0 = mslot[(pair[0], c)]
                            nc.vector.tensor_tensor(
                                out=pss[hx].rearrange("p a b -> p (a b)"),
                                in0=pss[hx].rearrange("p a b -> p (a b)"),
                                in1=maskt_sb[:, m0 : m0 + 2, :].rearrange(
                                    "p a b -> p (a b)"
                                ),
                                op=ALU.add,
                            )
                        else:
                            for n_, tk in enumerate(pair):
                                if needs_add[tk][c]:
                                    nc.vector.tensor_tensor(
                                        out=pss[hx][:, n_, :],
                                        in0=pss[hx][:, n_, :],
                                        in1=maskt_sb[:, mslot[(tk, c)], :],
                                        op=ALU.add,
                                    )
                    exp = []
                    for hx in range(2):
                        ex = ap_.tile([P, 2, QC], BF16, tag="ex", bufs=8, name="ex")
                        nc.scalar.activation(
                            out=ex.rearrange("p a b -> p (a b)")[:, :w],
                            in_=pss[hx].rearrange("p a b -> p (a b)")[:, :w],
                            func=AF.Exp,
                        )
                        exp.append(ex)
                    pairs.append(pair)
                    exps.append(exp)
                for hx in range(2):
                    nmm = 0
                    for pair, exp in zip(pairs, exps):
                        for n_, tk in enumerate(pair):
                            nc.tensor.matmul(
                                psa[hx][: D + 1, :],
                                vaug[:, g, tk, 2 * jl + hx, :],
                                exp[hx][:, n_, :],
                                start=(nmm == 0),
                                stop=(nmm == len(tks) - 1),
                            )
                            nmm += 1
                # normalize: one copy grabs both denominators (also orders the
                # DVE read after BOTH chains), reciprocal+apply per head.
                dens = ap_.tile([1, 2, QC], F32, tag="rec", bufs=3, name="dens")
                nc.vector.tensor_copy(out=_r(dens), in_=psa2[D : D + 1, :, :])
                for hx in range(2):
                    psb = psum.tile([P, 512], F32, tag="pj", bufs=2, name="psb")
                    nc.tensor.matmul(
                        psb[:64, :QC],
                        _r(ones64),
                        _r(dens[:, hx, :]),
                        start=True,
                        stop=True,
                    )
                    bcs = ap_.tile([64, QC], F32, tag="bcs", bufs=3, name="bcs")
                    nc.vector.reciprocal_approx_fast(out=bcs, in_=psb[:64, :QC])
                    ro = hx * 64
                    nc.vector.tensor_tensor(
                        out=OT[ro : ro + 64, p_, qsl],
                        in0=psa[hx][:D, :],
                        in1=bcs,
                        op=ALU.mult,
                    )

            def out_proj(i):
                for c2 in range(2):
                    cs = slice(c2 * 384, (c2 + 1) * 384)
                    ps = psum.tile([P, 512], F32, tag="pj", bufs=2, name="pso")
                    for ek in range(ET):
                        nc.tensor.matmul(
                            ps[:, :384],
                            OT[:, ek, i * P : (i + 1) * P],
                            wo_sb[:, ek, cs],
                            start=(ek == 0),
                            stop=(ek == ET - 1),
                        )
                    nc.vector.tensor_tensor(
                        out=x_h[:, i, cs], in0=ps[:, :384], in1=x_h[:, i, cs],
                        op=ALU.add,
                    )

            def ln2_pair(c):
                mvs = lp2.tile([P, 2, 2], F32, tag="mvs2", bufs=2, name="mvs2")
                vs = lp2.tile([P, 2], F32, tag="vs2", bufs=2, name="vs2")
                for sl in range(2):
                    ln_stats(lp2, x_h[:, 2 * c + sl, :], mvs, vs, sl)
                rs = newton_rsqrt(lp2, vs, 2)
                for sl in range(2):
                    i = 2 * c + sl
                    xn = ln_normalize(lp2, x_h[:, i, :], mvs, rs, sl)
                    ln_transposes(xn, h2T, i, g2c, b2cc)
                    # pre-add b2 so the m2 output lands with a single add
                    nc.vector.tensor_tensor(
                        out=x_h[:, i, :], in0=x_h[:, i, :], in1=b2b, op=ALU.add
                    )

            w1a = w1_t.ap().rearrange("(ko p) f -> p ko f", p=P)
            w2a = w2_t.ap().rearrange("(ko p) e -> p ko e", p=P)
            m1g = mp.tile([P, FT, 512], BF16, tag="m1g", bufs=1, name="m1g")

            def mlp_m1(sc):
                ssl = slice(sc * 512, (sc + 1) * 512)
                for w in range(6):
                    w1_sb = mp.tile([P, ET, 512], BF16, tag="w1s", bufs=2, name="w1s")
                    nc.sync.dma_start(w1_sb, w1a[:, :, w * 512 : (w + 1) * 512])
                    for ftl in range(4):
                        ft = 4 * w + ftl
                        ps = psum.tile([P, 512], F32, tag="pj", bufs=2, name="psm1")
                        for ek in range(ET):
                            nc.tensor.matmul(
                                ps,
                                w1_sb[:, ek, ftl * P : (ftl + 1) * P],
                                h2T[:, ek, ssl],
                                start=(ek == 0),
                                stop=(ek == ET - 1),
                            )
                        nc.scalar.activation(
                            out=m1g[:, ft, :],
                            in_=ps,
                            func=AF.Gelu,
                            bias=b1c[:, ft : ft + 1],
                            scale=1.0,
                        )

            def mlp_m2(sc):
                for c2 in range(3):
                    cs = slice(c2 * 256, (c2 + 1) * 256)
                    w2_sb = mp.tile([P, FT, 256], BF16, tag="w2s", bufs=2, name="w2s")
                    nc.sync.dma_start(w2_sb, w2a[:, :, cs])
                    for sl in range(4):
                        i = sc * 4 + sl
                        ps = psum.tile([P, 512], F32, tag="pj", bufs=2, name="psm2")
                        for fk in range(FT):
                            nc.tensor.matmul(
                                ps[:, :256],
                                m1g[:, fk, sl * P : (sl + 1) * P],
                                w2_sb[:, fk, :],
                                start=(fk == 0),
                                stop=(fk == FT - 1),
                            )
                        otile = mp.tile([P, 256], F32, tag="otile", bufs=3, name="ot")
                        nc.vector.tensor_tensor(
                            out=otile, in0=ps[:, :256], in1=x_h[:, i, cs], op=ALU.add
                        )
                        nc.sync.dma_start(outa[:, i, cs], otile)

            # out-proj/LN2 for chunk c is emitted AFTER chunk c+1's attention
            # units: the PE instruction stream is static, so this guarantees
            # ready matmul work while chunk c's softmax tails drain.
            for c in range(NQC):
                for p_ in range(NP_):
                    attn_unit(p_, c)
                if c >= 1:
                    out_proj(2 * (c - 1))
                    out_proj(2 * c - 1)
                    ln2_pair(c - 1)
            mlp_m1(0)  # ready work for the PE while chunk 3's tails drain
            out_proj(2 * NQC - 2)
            out_proj(2 * NQC - 1)
            ln2_pair(NQC - 1)
            mlp_m2(0)
            mlp_m1(1)
            mlp_m2(1)

    nc.compile()
    return nc


_CACHE = {}


def _get_nc(kept, needs_add):
    key = kept.tobytes() + needs_add.tobytes()
    if key not in _CACHE:
        _CACHE[key] = build(kept, needs_add)
    return _CACHE[key]


def prepare(inputs):
    """Host-side prep: mask structure + transposed clamped bf16 slabs,
    bf16 weights (SCALE folded into wq/bq). Returns (nc, in_maps)."""
    inp = {k: np.asarray(v) for k, v in inputs.items()}
    mask = np.ascontiguousarray(np.asarray(inp["mask"], np.float32))  # [B,1,S,S]

    kept = np.zeros((ST, NQC), bool)
    needs_add = np.zeros((ST, NQC), bool)
    for tk in range(ST):
        for c in range(NQC):
            blk = mask[:, 0, c * QC : (c + 1) * QC, tk * P : (tk + 1) * P]
            kept[tk, c] = bool(np.any(blk > -1e8))
            needs_add[tk, c] = kept[tk, c] and bool(np.any(blk != 0.0))

    nc = _get_nc(kept, needs_add)

    nslab = int(needs_add.sum())
    maskt = None
    if nslab:
        slabs = []
        for c in range(NQC):
            for tk in range(ST):
                if needs_add[tk][c]:
                    blk = mask[:, 0, c * QC : (c + 1) * QC, tk * P : (tk + 1) * P]
                    slabs.append(np.clip(blk, MASK_CLAMP, None).transpose(0, 2, 1))
        # order must match mslot (c-major outer, tk inner)
        maskt = np.ascontiguousarray(
            np.stack(slabs, axis=1).astype(NPBF16)  # [B, nslab, 128, 256]
            .transpose(0, 2, 1, 3)                  # [B, 128, nslab, 256]
        )

    f32 = lambda a: np.ascontiguousarray(np.asarray(a, np.float32))
    bf = lambda a: np.ascontiguousarray(np.asarray(a, np.float32).astype(NPBF16))
    shared = {
        "ln1_g": f32(inp["ln1_g"]), "ln1_b": f32(inp["ln1_b"]),
        "ln2_g": f32(inp["ln2_g"]), "ln2_b": f32(inp["ln2_b"]),
        "bq": f32(np.asarray(inp["bq"], np.float32) * SCALE),
        "bk": f32(inp["bk"]), "bv": f32(inp["bv"]), "bo": f32(inp["bo"]),
        "b1": f32(inp["b1"]), "b2": f32(inp["b2"]),
        "wq": bf(np.asarray(inp["wq"], np.float32) * SCALE),
        "wk": bf(inp["wk"]), "wv": bf(inp["wv"]), "wo": bf(inp["wo"]),
        "w1": bf(inp["w1"]), "w2": bf(inp["w2"]),
    }
    x = f32(inp["x"])
    in_maps = []
    for b in range(B):
        m = {"x": x[b], **shared}
        if maskt is not None:
            m["maskt"] = maskt[b]
        in_maps.append(m)
    return nc, in_maps


def kernel(**inputs) -> np.ndarray:
    nc, in_maps = prepare(inputs)
    res = run_bass_kernel_spmd(nc, in_maps, core_ids=list(range(B)))
    return np.stack([res.results[b]["out"] for b in range(B)], axis=0)


if __name__ == "__main__":
    # smoke build with the causal structure
    ck = np.zeros((ST, NQC), bool)
    ca = np.zeros((ST, NQC), bool)
    for tk in range(ST):
        for c in range(NQC):
            lo, hi = c * QC, (c + 1) * QC - 1
            ck[tk, c] = tk * P <= hi
            ca[tk, c] = ck[tk, c] and not (tk * P + P - 1 <= lo)
    build(ck, ca)
    print("build ok")


# revision 28
# speedup vs baseline: 1.2568x; 1.2568x over previous
"""CLIP encoder layer (LN -> causal MHA -> residual -> LN -> GELU MLP -> residual)
as a Bass/Tile kernel for Trainium2, data-parallel over batch across 8 NeuronCores.

v2 layout strategy per core (one batch element):
  - matmul path in bf16 (full PE rate, FWL fast weight loads, half the HBM
    traffic); residuals/LN/softmax math in fp32; PSUM accumulation fp32.
  - mask prep done on HOST: transposed, clamped, bf16 slabs only for blocks
    that are partially masked; fully-masked 128x256 score blocks are skipped,
    fully-live blocks need no mask add at all.
  - h kept natural fp32 (residual+LN) and normalized-transposed bf16 (h1T/h2T).
  - attention in scoresT[t_key, s_query] layout, 256-wide query chunks;
    2 heads share the PE via row tiling (kT/qT for the head pair live at
    partitions 0:64 / 64:128 -> tile_position (0,0)/(64,0) auto-derived);
    score blocks for two key tiles share one PSUM bank so exp runs as one
    [128,512] ACT op; softmax denominator = ones column appended to V
    (row 64 of the AV psum); reciprocal via the fast DVE approximation,
    broadcast to [64, q] with a tiny PE matmul.
  - bo/b2 biases pre-added into the residual so projection outputs land with
    a single fused add.
"""

import numpy as np
import ml_dtypes
from contextlib import ExitStack

import concourse.bass as bass
import concourse.mybir as mybir
import concourse.tile as tile
from concourse import bacc
from concourse.bass_utils import run_bass_kernel_spmd
from concourse.masks import make_identity

AF = mybir.ActivationFunctionType
ALU = mybir.AluOpType
F32 = mybir.dt.float32
F32R = mybir.dt.float32r
BF16 = mybir.dt.bfloat16
NPBF16 = ml_dtypes.bfloat16

B, S, E, H, D, F = 8, 1024, 768, 12, 64, 3072
P = 128
ST = S // P            # 8 token tiles
ET = E // P            # 6 embed tiles
FT = F // P            # 24 ffn tiles
QC = 256               # attention query-chunk width
NQC = S // QC          # 4
NP_ = 6                # head pairs
SCALE = float(D) ** -0.5
EPS = 1e-5
MASK_CLAMP = -80.0


def _r(ap):
    """Reinterpret an fp32 AP as float32r for full-rate PE matmuls."""
    return ap.bitcast(F32R)


def _bcast_dma(nc, dst, src_ap):
    """DMA a 1-D DRAM vector to [P, n] SBUF, broadcast along partitions."""
    bsrc = bass.AP(
        tensor=src_ap.tensor, offset=src_ap.offset, ap=[[0, P]] + list(src_ap.ap)
    )
    nc.gpsimd.dma_start(out=dst, in_=bsrc)


def build(kept, needs_add):
    """kept[tk][c]: [128-key x 256-query] score block live for any batch.
    needs_add[tk][c]: block needs an additive mask slab (mixed/partial)."""
    kept = np.asarray(kept, bool)
    needs_add = np.asarray(needs_add, bool)
    nslab = int(needs_add.sum())
    mslot = {}
    for c in range(NQC):
        for tk in range(ST):
            if needs_add[tk][c]:
                mslot[(tk, c)] = len(mslot)

    nc = bacc.Bacc("TRN2", target_bir_lowering=False, debug=False, num_devices=8)

    x_t = nc.dram_tensor("x", [S, E], F32, kind="ExternalInput")
    maskt_t = (
        nc.dram_tensor("maskt", [P, nslab, QC], BF16, kind="ExternalInput")
        if nslab
        else None
    )
    # small per-channel vectors pre-packed on host into [P, 60]:
    # bq*SCALE | bk | ln1_g | ln1_b | ln2_g | ln2_b (6 cols each), b1 (24)
    vecs_t = nc.dram_tensor("vecs", [P, 60], F32, kind="ExternalInput")
    names_1d = ["bv", "bo", "b2"]
    v1 = {n: nc.dram_tensor(n, [E], F32, kind="ExternalInput") for n in names_1d}
    wq_t = nc.dram_tensor("wq", [E, E], BF16, kind="ExternalInput")
    wk_t = nc.dram_tensor("wk", [E, E], BF16, kind="ExternalInput")
    wv_t = nc.dram_tensor("wv", [E, E], BF16, kind="ExternalInput")
    wo_t = nc.dram_tensor("wo", [E, E], BF16, kind="ExternalInput")
    w1_t = nc.dram_tensor("w1", [E, F], BF16, kind="ExternalInput")
    w2_t = nc.dram_tensor("w2", [F, E], BF16, kind="ExternalInput")
    out_t = nc.dram_tensor("out", [S, E], F32, kind="ExternalOutput")

    xa = x_t.ap().rearrange("(n p) e -> p n e", p=P)          # [P, ST, E]
    outa = out_t.ap().rearrange("(n p) e -> p n e", p=P)

    with tile.TileContext(nc) as tc, ExitStack() as top, nc.allow_low_precision(
        reason="bf16 matmul path; accumulation stays fp32 in PSUM"
    ):
        persist = top.enter_context(tc.tile_pool(name="persist", bufs=1))
        psum = top.enter_context(tc.tile_pool(name="psum", bufs=1, space="PSUM"))

        # ---- persistent tiles + small constants ----
        x_h = persist.tile([P, ST, E], F32, name="x_h")       # x, then residual h
        OT = persist.tile([P, ET, S], BF16, name="OT")        # attn out (unnormed->normed), T
        identity = persist.tile([P, P], BF16, name="identity")
        make_identity(nc, identity)
        eps_t = persist.tile([P, 1], F32, name="eps_t")
        nc.vector.memset(eps_t, EPS)
        ones64 = persist.tile([1, 64], F32, name="ones64")
        nc.vector.memset(ones64, 1.0)

        for i in range(ST):  # per-tile so LN1 starts before the full x lands
            nc.sync.dma_start(x_h[:, i, :], xa[:, i, :])

        vecs = persist.tile([P, 60], F32, name="vecs")
        nc.gpsimd.dma_start(vecs, vecs_t.ap())
        bqs, bks = vecs[:, 0:6], vecs[:, 6:12]
        g1c, b1cc = vecs[:, 12:18], vecs[:, 18:24]
        g2c, b2cc = vecs[:, 24:30], vecs[:, 30:36]
        b1c = vecs[:, 36:60]
        bv_b = persist.tile([P, E], F32, name="bv_b")
        bo_b = persist.tile([P, E], F32, name="bo_b")
        b2b = persist.tile([P, E], F32, name="b2b")
        _bcast_dma(nc, bv_b, v1["bv"].ap())
        _bcast_dma(nc, bo_b, v1["bo"].ap())
        _bcast_dma(nc, b2b, v1["b2"].ap())

        wo_sb = persist.tile([P, ET, E], BF16, name="wo_sb")
        nc.scalar.dma_start(wo_sb, wo_t.ap().rearrange("(ko p) m -> p ko m", p=P))

        h1T = persist.tile([P, ET, S], BF16, tag="hT", name="h1T")

        U32 = mybir.dt.uint32
        magic = persist.tile([P, ST], U32, name="magic")
        nc.vector.memset(magic, 0x5F3759DF)

        def newton_rsqrt(pool, v, n):
            """DVE-only rstd = v^-1/2 on a [P, n] fp32 tile (quake seed + 2
            Newton iterations, rel err ~1e-6). Keeps ACT free of Ln/Sqrt so
            the exp table stays loaded for the whole attention phase."""
            y = pool.tile([P, n], F32, tag="lnrsy", bufs=2, name="rsy")
            t = pool.tile([P, n], F32, tag="lnrst", bufs=2, name="rst")
            # seed: bits(y) = 0x5F3759DF - (bits(v) >> 1)  (as ~x + C+1)
            nc.vector.tensor_scalar(
                out=y.bitcast(U32),
                in0=v.bitcast(U32),
                scalar1=1,
                scalar2=None,
                op0=ALU.logical_shift_right,
            )
            nc.vector.tensor_tensor(
                out=y.bitcast(U32),
                in0=magic[:, :n],
                in1=y.bitcast(U32),
                op=ALU.subtract,
            )
            for _ in range(2):
                nc.vector.tensor_tensor(out=t, in0=y, in1=y, op=ALU.mult)
                nc.vector.tensor_tensor(out=t, in0=t, in1=v, op=ALU.mult)
                nc.vector.tensor_scalar(
                    out=t, in0=t, scalar1=-0.5, scalar2=1.5, op0=ALU.mult, op1=ALU.add
                )
                nc.vector.tensor_tensor(out=y, in0=y, in1=t, op=ALU.mult)
            return y

        def ln_stats(pool, x_slice, mvs, vs, sl):
            """bn stats for one token tile; mean into mvs[:, sl, :], var+eps
            into vs[:, sl]."""
            xr = x_slice.rearrange("p (n s) -> p n s", s=256)
            stats = pool.tile([P, 3, 6], F32, tag="lnstats", bufs=4, name="stats")
            for sg in range(3):
                nc.vector.bn_stats(out=stats[:, sg, :], in_=xr[:, sg, :])
            nc.vector.bn_aggr(out=mvs[:, sl, :], in_=stats)
            nc.vector.tensor_scalar_add(vs[:, sl : sl + 1], mvs[:, sl, 1:2], EPS)

        def ln_normalize(pool, x_slice, mvs, rs, sl):
            xn = pool.tile([P, E], BF16, tag="lnxn", bufs=3, name="xn")
            nc.vector.tensor_scalar(
                out=xn,
                in0=x_slice,
                scalar1=mvs[:, sl, 0:1],
                scalar2=rs[:, sl : sl + 1],
                op0=ALU.subtract,
                op1=ALU.mult,
            )
            return xn

        def ln_transposes(xn, dstT, i, gc, bc):
            for j in range(ET):
                pt = psum.tile([P, P], BF16, tag="pss", bufs=3, name="pt")
                ptv = pt
                nc.tensor.transpose(ptv, xn[:, j * P : (j + 1) * P], identity)
                nc.scalar.activation(
                    out=dstT[:, j, i * P : (i + 1) * P],
                    in_=ptv,
                    func=AF.Identity,
                    bias=bc[:, j : j + 1],
                    scale=gc[:, j : j + 1],
                )

        # ---------------- LN1 + transpose to h1T ----------------
        with tc.tile_pool(name="ln1p", bufs=1) as lp:
            mvs = lp.tile([P, ST, 2], F32, name="mvs1")
            vs = lp.tile([P, ST], F32, name="vs1")
            for i in range(ST):
                ln_stats(lp, x_h[:, i, :], mvs, vs, i)
            rs = newton_rsqrt(lp, vs, ST)
            for i in range(ST):
                xn = ln_normalize(lp, x_h[:, i, :], mvs, rs, i)
                ln_transposes(xn, h1T, i, g1c, b1cc)

        # ---------------- QKV projections (all 3 groups) ----------------
        qT = persist.tile([P, 3, 2, S], BF16, name="qT")
        kT = persist.tile([P, 3, 2, S], BF16, name="kT")
        vaug = persist.tile([P, 3, ST, 4, D + 1], BF16, name="vaug")
        nc.gpsimd.memset(vaug[:, :, :, :, D : D + 1], 1.0)

        with tc.tile_pool(name="wqkv", bufs=1) as wp:
            wq_sb = wp.tile([P, ET, E], BF16, name="wq_sb")
            wk_sb = wp.tile([P, ET, E], BF16, name="wk_sb")
            wv_sb = wp.tile([P, ET, E], BF16, name="wv_sb")
            nc.scalar.dma_start(wq_sb, wq_t.ap().rearrange("(ko p) m -> p ko m", p=P))
            nc.scalar.dma_start(wk_sb, wk_t.ap().rearrange("(ko p) m -> p ko m", p=P))
            nc.gpsimd.dma_start(wv_sb, wv_t.ap().rearrange("(ko p) m -> p ko m", p=P))

            for g in range(3):
                for w_sb, dstT, bias in ((wq_sb, qT, bqs), (wk_sb, kT, bks)):
                    for jl in range(2):
                        jj = 2 * g + jl
                        for sc2 in range(2):
                            ps = psum.tile([P, 512], F32, tag="pj", bufs=2, name="psq")
                            for ek in range(ET):
                                nc.tensor.matmul(
                                    ps,
                                    w_sb[:, ek, jj * P : (jj + 1) * P],
                                    h1T[:, ek, sc2 * 512 : (sc2 + 1) * 512],
                                    start=(ek == 0),
                                    stop=(ek == ET - 1),
                                )
                            nc.scalar.activation(
                                out=dstT[:, g, jl, sc2 * 512 : (sc2 + 1) * 512],
                                in_=ps,
                                func=AF.Identity,
                                bias=bias[:, jj : jj + 1],
                                scale=1.0,
                            )
                gsl = slice(g * 256, (g + 1) * 256)
                bvr = bv_b[:, gsl].rearrange("p (h d) -> p h d", d=D)
                for i in range(ST):
                    ps = psum.tile([P, 512], F32, tag="pj", bufs=2, name="psv")
                    for ek in range(ET):
                        nc.tensor.matmul(
                            ps[:, :256],
                            h1T[:, ek, i * P : (i + 1) * P],
                            wv_sb[:, ek, gsl],
                            start=(ek == 0),
                            stop=(ek == ET - 1),
                        )
                    nc.vector.tensor_tensor(
                        out=vaug[:, g, i, :, 0:D],
                        in0=ps[:, :256].rearrange("p (h d) -> p h d", d=D),
                        in1=bvr,
                        op=ALU.add,
                    )

        # ---------------- attention + out-proj + LN2 + MLP (interleaved) ----
        h2T = persist.tile([P, ET, S], BF16, tag="hT", name="h2T")
        kept_by_c = {c: [tk for tk in range(ST) if kept[tk][c]] for c in range(NQC)}

        with tc.tile_pool(name="attnp", bufs=1) as ap_, tc.tile_pool(
            name="mlpp", bufs=1
        ) as mp, tc.tile_pool(name="ln2p", bufs=1) as lp2:
            maskt_sb = None
            if nslab:
                maskt_sb = ap_.tile([P, nslab, QC], BF16, name="maskt_sb")
                nc.gpsimd.dma_start(maskt_sb, maskt_t.ap())

            # pre-add bo into the residual (out-proj then lands with one add)
            for i in range(ST):
                nc.vector.tensor_tensor(
                    out=x_h[:, i, :], in0=x_h[:, i, :], in1=bo_b, op=ALU.add
                )

            def attn_unit(p_, c):
                """One head pair x one 256-query chunk."""
                g, jl = divmod(p_, 2)
                tks = kept_by_c[c]
                if not tks:
                    return
                qsl = slice(c * QC, (c + 1) * QC)
                # both heads' AV accumulators share one PSUM bank so three
                # units can be in flight across the 8-bank budget. start=True
                # clears has_written BANK-wide, so the two accumulation chains
                # must NOT interleave: head A's chain runs fully, then head
                # B's (B's start only clears bits; A's data stays readable).
                psa2 = psum.tile([P, 2, QC], F32, tag="pav", bufs=3, name="psa")
                psa = {hx: psa2[:, hx, :] for hx in range(2)}
                pairs = []
                exps = []
                npairs = (len(tks) + 1) // 2
                for tp in range(npairs):
                    pair = tks[2 * tp : 2 * tp + 2]
                    w = len(pair) * QC
                    pss = {}
                    for hx in range(2):
                        ro = hx * 64
                        pss[hx] = psum.tile(
                            [P, 2, QC], F32, tag="pss", bufs=3, name="pss"
                        )
                        for n_, tk in enumerate(pair):
                            nc.tensor.matmul(
                                pss[hx][:, n_, :],
                                kT[ro : ro + 64, g, jl, tk * P : (tk + 1) * P],
                                qT[ro : ro + 64, g, jl, qsl],
                                start=True,
                                stop=True,
                            )
                    adds = [needs_add[tk][c] for tk in pair]
                    merged = (
                        len(pair) == 2
                        and all(adds)
                        and mslot[(pair[1], c)] == mslot[(pair[0], c)] + 1
                    )
                    for hx in range(2):
                        if merged:
                            m0 = mslot[(pair[0], c)]
                            nc.vector.tensor_tensor(
                                out=pss[hx].rearrange("p a b -> p (a b)"),
                                in0=pss[hx].rearrange("p a b -> p (a b)"),
                                in1=maskt_sb[:, m0 : m0 + 2, :].rearrange(
                                    "p a b -> p (a b)"
                                ),
                                op=ALU.add,
                            )
                        else:
                            for n_, tk in enumerate(pair):
                                if needs_add[tk][c]:
                                    nc.vector.tensor_tensor(
                                        out=pss[hx][:, n_, :],
                                        in0=pss[hx][:, n_, :],
                                        in1=maskt_sb[:, mslot[(tk, c)], :],
                                        op=ALU.add,
                                    )
                    exp = []
                    for hx in range(2):
                        ex = ap_.tile([P, 2, QC], BF16, tag="ex", bufs=8, name="ex")
                        nc.scalar.activation(
                            out=ex.rearrange("p a b -> p (a b)")[:, :w],
                            in_=pss[hx].rearrange("p a b -> p (a b)")[:, :w],
                            func=AF.Exp,
                        )
                        exp.append(ex)
                    pairs.append(pair)
                    exps.append(exp)
                for hx in range(2):
                    nmm = 0
                    for pair, exp in zip(pairs, exps):
                        for n_, tk in enumerate(pair):
                            nc.tensor.matmul(
                                psa[hx][: D + 1, :],
                                vaug[:, g, tk, 2 * jl + hx, :],
                                exp[hx][:, n_, :],
                                start=(nmm == 0),
                                stop=(nmm == len(tks) - 1),
                            )
                            nmm += 1
                # normalize: one copy grabs both denominators (also orders the
                # DVE read after BOTH chains), reciprocal+apply per head.
                dens = ap_.tile([1, 2, QC], F32, tag="rec", bufs=3, name="dens")
                nc.vector.tensor_copy(out=_r(dens), in_=psa2[D : D + 1, :, :])
                for hx in range(2):
                    psb = psum.tile([P, 2, QC], F32, tag="pav", bufs=3, name="psb")
                    nc.tensor.matmul(
                        psb[:64, 0, :],
                        _r(ones64),
                        _r(dens[:, hx, :]),
                        start=True,
                        stop=True,
                    )
                    bcs = ap_.tile([64, QC], F32, tag="bcs", bufs=3, name="bcs")
                    nc.vector.reciprocal_approx_fast(out=bcs, in_=psb[:64, 0, :])
                    ro = hx * 64
                    nc.vector.tensor_tensor(
                        out=OT[ro : ro + 64, p_, qsl],
                        in0=psa[hx][:D, :],
                        in1=bcs,
                        op=ALU.mult,
                    )

            def out_proj(i):
                for c2 in range(2):
                    cs = slice(c2 * 384, (c2 + 1) * 384)
                    ps = psum.tile([P, 512], F32, tag="pj", bufs=2, name="pso")
                    for ek in range(ET):
                        nc.tensor.matmul(
                            ps[:, :384],
                            OT[:, ek, i * P : (i + 1) * P],
                            wo_sb[:, ek, cs],
                            start=(ek == 0),
                            stop=(ek == ET - 1),
                        )
                    nc.vector.tensor_tensor(
                        out=x_h[:, i, cs], in0=ps[:, :384], in1=x_h[:, i, cs],
                        op=ALU.add,
                    )

            def ln2_pair(c):
                mvs = lp2.tile([P, 2, 2], F32, tag="mvs2", bufs=2, name="mvs2")
                vs = lp2.tile([P, 2], F32, tag="vs2", bufs=2, name="vs2")
                for sl in range(2):
                    ln_stats(lp2, x_h[:, 2 * c + sl, :], mvs, vs, sl)
                rs = newton_rsqrt(lp2, vs, 2)
                for sl in range(2):
                    i = 2 * c + sl
                    xn = ln_normalize(lp2, x_h[:, i, :], mvs, rs, sl)
                    ln_transposes(xn, h2T, i, g2c, b2cc)
                    # pre-add b2 so the m2 output lands with a single add
                    nc.vector.tensor_tensor(
                        out=x_h[:, i, :], in0=x_h[:, i, :], in1=b2b, op=ALU.add
                    )

            w1a = w1_t.ap().rearrange("(ko p) f -> p ko f", p=P)
            w2a = w2_t.ap().rearrange("(ko p) e -> p ko e", p=P)
            m1g = mp.tile([P, FT, 512], BF16, tag="m1g", bufs=1, name="m1g")

            def mlp_m1(sc):
                ssl = slice(sc * 512, (sc + 1) * 512)
                for w in range(6):
                    w1_sb = mp.tile([P, ET, 512], BF16, tag="w1s", bufs=2, name="w1s")
                    nc.sync.dma_start(w1_sb, w1a[:, :, w * 512 : (w + 1) * 512])
                    for ftl in range(4):
                        ft = 4 * w + ftl
                        ps = psum.tile([P, 512], F32, tag="pj", bufs=2, name="psm1")
                        for ek in range(ET):
                            nc.tensor.matmul(
                                ps,
                                w1_sb[:, ek, ftl * P : (ftl + 1) * P],
                                h2T[:, ek, ssl],
                                start=(ek == 0),
                                stop=(ek == ET - 1),
                            )
                        nc.scalar.activation(
                            out=m1g[:, ft, :],
                            in_=ps,
                            func=AF.Gelu,
                            bias=b1c[:, ft : ft + 1],
                            scale=1.0,
                        )

            def mlp_m2(sc):
                for c2 in range(3):
                    cs = slice(c2 * 256, (c2 + 1) * 256)
                    w2_sb = mp.tile([P, FT, 256], BF16, tag="w2s", bufs=2, name="w2s")
                    nc.sync.dma_start(w2_sb, w2a[:, :, cs])
                    for sl in range(4):
                        i = sc * 4 + sl
                        ps = psum.tile([P, 512], F32, tag="pj", bufs=2, name="psm2")
                        for fk in range(FT):
                            nc.tensor.matmul(
                                ps[:, :256],
                                m1g[:, fk, sl * P : (sl + 1) * P],
                                w2_sb[:, fk, :],
                                start=(fk == 0),
                                stop=(fk == FT - 1),
                            )
                        otile = mp.tile([P, 256], F32, tag="otile", bufs=3, name="ot")
                        nc.vector.tensor_tensor(
                            out=otile, in0=ps[:, :256], in1=x_h[:, i, cs], op=ALU.add
                        )
                        nc.sync.dma_start(outa[:, i, cs], otile)

            # out-proj/LN2 for chunk c is emitted AFTER chunk c+1's attention
            # units: the PE instruction stream is static, so this guarantees
            # ready matmul work while chunk c's softmax tails drain.
            for c in range(NQC):
                for p_ in range(NP_):
                    attn_unit(p_, c)
                if c >= 1:
                    out_proj(2 * (c - 1))
                    out_proj(2 * c - 1)
                    ln2_pair(c - 1)
            mlp_m1(0)  # ready work for the PE while chunk 3's tails drain
            out_proj(2 * NQC - 2)
            out_proj(2 * NQC - 1)
            ln2_pair(NQC - 1)
            mlp_m2(0)
            mlp_m1(1)
            mlp_m2(1)

    nc.compile()
    return nc


_CACHE = {}


def _get_nc(kept, needs_add):
    key = kept.tobytes() + needs_add.tobytes()
    if key not in _CACHE:
        _CACHE[key] = build(kept, needs_add)
    return _CACHE[key]


def prepare(inputs):
    """Host-side prep: mask structure + transposed clamped bf16 slabs,
    bf16 weights (SCALE folded into wq/bq). Returns (nc, in_maps)."""
    inp = {k: np.asarray(v) for k, v in inputs.items()}
    mask = np.ascontiguousarray(np.asarray(inp["mask"], np.float32))  # [B,1,S,S]

    kept = np.zeros((ST, NQC), bool)
    needs_add = np.zeros((ST, NQC), bool)
    for tk in range(ST):
        for c in range(NQC):
            blk = mask[:, 0, c * QC : (c + 1) * QC, tk * P : (tk + 1) * P]
            kept[tk, c] = bool(np.any(blk > -1e8))
            needs_add[tk, c] = kept[tk, c] and bool(np.any(blk != 0.0))

    nc = _get_nc(kept, needs_add)

    nslab = int(needs_add.sum())
    maskt = None
    if nslab:
        slabs = []
        for c in range(NQC):
            for tk in range(ST):
                if needs_add[tk][c]:
                    blk = mask[:, 0, c * QC : (c + 1) * QC, tk * P : (tk + 1) * P]
                    slabs.append(np.clip(blk, MASK_CLAMP, None).transpose(0, 2, 1))
        # order must match mslot (c-major outer, tk inner)
        maskt = np.ascontiguousarray(
            np.stack(slabs, axis=1).astype(NPBF16)  # [B, nslab, 128, 256]
            .transpose(0, 2, 1, 3)                  # [B, 128, nslab, 256]
        )

    f32 = lambda a: np.ascontiguousarray(np.asarray(a, np.float32))
    bf = lambda a: np.ascontiguousarray(np.asarray(a, np.float32).astype(NPBF16))
    col = lambda a: np.asarray(a, np.float32).reshape(-1, P).T  # [(o p)] -> [P, o]
    vecs = np.concatenate(
        [
            col(np.asarray(inp["bq"], np.float32) * SCALE),
            col(inp["bk"]), col(inp["ln1_g"]), col(inp["ln1_b"]),
            col(inp["ln2_g"]), col(inp["ln2_b"]), col(inp["b1"]),
        ],
        axis=1,
    )
    shared = {
        "vecs": f32(vecs),
        "bv": f32(inp["bv"]), "bo": f32(inp["bo"]), "b2": f32(inp["b2"]),
        "wq": bf(np.asarray(inp["wq"], np.float32) * SCALE),
        "wk": bf(inp["wk"]), "wv": bf(inp["wv"]), "wo": bf(inp["wo"]),
        "w1": bf(inp["w1"]), "w2": bf(inp["w2"]),
    }
    x = f32(inp["x"])
    in_maps = []
    for b in range(B):
        m = {"x": x[b], **shared}
        if maskt is not None:
            m["maskt"] = maskt[b]
        in_maps.append(m)
    return nc, in_maps


def kernel(**inputs) -> np.ndarray:
    nc, in_maps = prepare(inputs)
    res = run_bass_kernel_spmd(nc, in_maps, core_ids=list(range(B)))
    return np.stack([res.results[b]["out"] for b in range(B)], axis=0)


if __name__ == "__main__":
    # smoke build with the causal structure
    ck = np.zeros((ST, NQC), bool)
    ca = np.zeros((ST, NQC), bool)
    for tk in range(ST):
        for c in range(NQC):
            lo, hi = c * QC, (c + 1) * QC - 1
            ck[tk, c] = tk * P <= hi
            ca[tk, c] = ck[tk, c] and not (tk * P + P - 1 <= lo)
    build(ck, ca)
    print("build ok")
